# revision 2
# baseline (speedup 1.0000x reference)
"""BitLinear (2-bit packed weights) on 8 Trainium2 NeuronCores.

- Data-parallel over tokens (2048 rows/core); weight unpacked host-side
  to bf16 [in, out] and replicated.
- PE runs ONLY the 1024 main matmuls per core; activation transposes run
  on the DMA xbar (one batched InstDmaTransposeAnt per m-tile, bit-exact).
- Modulo ring pipeline, 5 tiles deep (front((t+5)%16) then tail(t)); the
  ring closes seamlessly across For_i timing iterations.
- x-in prefetch alone on the SP hwdge queue; xbar + y-out on the ACT
  queue. PSUM double-buffered (4 tags x 2 bufs = 8 banks). Dequant+bias
  fused in one DVE scalar_tensor_tensor per chunk.

Math is bit-faithful to the jax reference (magic-number round-half-even,
exact int products in bf16 PE matmul with f32 PSUM accumulation)."""

from contextlib import ExitStack

import numpy as np
import ml_dtypes

import concourse.bass as bass
import concourse.mybir as mybir
import concourse.tile as tile
from concourse import bacc
from concourse.bass_utils import run_bass_kernel_spmd

P = 128
D = 2048               # in_features
O = 2048               # out_features
N_CORES = 8
B, S = 4, 4096
M_TOTAL = B * S
M_CORE = M_TOTAL // N_CORES   # 2048
NK = D // P            # 16 contraction blocks
O_CHUNK = 512
N_OCH = O // O_CHUNK   # 4
MAGIC = 12582912.0     # 1.5 * 2^23
QP = 127.0
AHEAD = 2              # transpose pipeline depth (tiles)


def build_nc(m_core=M_CORE, repeats=1, variant="a5"):
    ahead = {"a4": 4, "a5": 5, "a8": 8, "a6": 6}.get(variant, AHEAD)
    m_tiles = m_core // P
    nc = bacc.Bacc(None)
    x = nc.declare_dram_parameter("x", [m_core, D], mybir.dt.float32, isOutput=False)
    wT = nc.declare_dram_parameter("wT", [D, O], mybir.dt.bfloat16, isOutput=False)
    bias = nc.declare_dram_parameter("bias", [O], mybir.dt.float32, isOutput=False)
    ws = nc.declare_dram_parameter("ws", [1], mybir.dt.float32, isOutput=False)
    y = nc.declare_dram_parameter("y", [m_core, O], mybir.dt.float32, isOutput=True)

    with ExitStack() as ctx:
        tc = ctx.enter_context(tile.TileContext(nc))
        consts = ctx.enter_context(tc.tile_pool(name="consts", bufs=1))
        xpool = ctx.enter_context(tc.tile_pool(name="xin", bufs=4))
        qpool = ctx.enter_context(tc.tile_pool(name="quant", bufs=3))
        tppool = ctx.enter_context(tc.tile_pool(name="xqt", bufs=ahead + 2))
        spool = ctx.enter_context(tc.tile_pool(name="stats", bufs=ahead + 4))
        opool = ctx.enter_context(tc.tile_pool(name="yout", bufs=3))
        psy = ctx.enter_context(tc.tile_pool(name="psy", bufs=2, space="PSUM"))

        bias_sb = consts.tile([P, O], mybir.dt.float32)
        nc.sync.dma_start(bias_sb[:], bias[None, :].to_broadcast((P, O)))
        ws_sb = consts.tile([P, 1], mybir.dt.float32)
        nc.sync.dma_start(ws_sb[:], ws[None, :].to_broadcast((P, 1)))
        w_sb = consts.tile([P, NK, O], mybir.dt.bfloat16)
        nc.sync.dma_start(w_sb[:], wT.rearrange("(k p) o -> p k o", p=P))

        x3 = x.rearrange("(t p) d -> t p d", p=P)
        y3 = y.rearrange("(t p) o -> t p o", p=P)

        def emit_front(t):
            """DMA x in, quantize, xbar-transpose -> (xqT tile, rden)."""
            xt = xpool.tile([P, D], mybir.dt.float32, tag="xin")
            nc.sync.dma_start(xt[:], x3[t])

            amax = spool.tile([P, 1], mybir.dt.float32, tag="amax")
            nc.vector.reduce_max(
                amax[:], xt[:], axis=mybir.AxisListType.X,
                apply_absolute_value=True,
            )
            nc.vector.tensor_scalar_max(amax[:], amax[:], 1e-5)
            ramax = spool.tile([P, 1], mybir.dt.float32, tag="ramax")
            nc.vector.reciprocal(ramax[:], amax[:])
            scl = spool.tile([P, 1], mybir.dt.float32, tag="scl")
            nc.vector.tensor_scalar_mul(scl[:], ramax[:], QP)
            den = spool.tile([P, 1], mybir.dt.float32, tag="den")
            nc.vector.tensor_tensor(
                den[:], ws_sb[:], scl[:], mybir.AluOpType.mult
            )
            rden = spool.tile([P, 1], mybir.dt.float32, tag="rden")
            nc.vector.reciprocal(rden[:], den[:])

            # xq = round_half_even(x * scale): DVE mult+MAGIC, ACT -MAGIC->bf16
            t1 = qpool.tile([P, D], mybir.dt.float32, tag="t1")
            nc.vector.tensor_scalar(
                t1[:], xt[:], scl[:], MAGIC,
                op0=mybir.AluOpType.mult, op1=mybir.AluOpType.add,
            )
            xq = qpool.tile([P, D], mybir.dt.bfloat16, tag="xq")
            nc.scalar.activation(
                xq[:], t1[:], mybir.ActivationFunctionType.Copy,
                bias=-MAGIC, scale=1.0,
            )

            # one batched xbar DMA: st[p, k, m] = xq[m, 128k + p]
            # (ACT hwdge queue: keeps the SP queue free for x-in prefetch)
            st = tppool.tile([P, NK, P], mybir.dt.bfloat16, tag="xqT")
            nc.scalar.dma_start_transpose(st[:], xq[:])
            return st, rden

        def emit_tail(t, st, rden):
            ys = [
                psy.tile([P, O_CHUNK], mybir.dt.float32,
                         tag=f"psy{j}", name=f"psy{j}")
                for j in range(N_OCH)
            ]
            for k in range(NK):
                for j in range(N_OCH):
                    nc.tensor.matmul(
                        ys[j][:], st[:, k, :],
                        w_sb[:, k, bass.ts(j, O_CHUNK)],
                        start=(k == 0), stop=(k == NK - 1),
                    )
            yt = opool.tile([P, O], mybir.dt.float32, tag="yt")
            for j in range(N_OCH):
                # fused dequant+bias on DVE: yt = (psum * rden) + bias
                nc.vector.scalar_tensor_tensor(
                    yt[:, bass.ts(j, O_CHUNK)], ys[j][:], rden[:],
                    bias_sb[:, bass.ts(j, O_CHUNK)],
                    op0=mybir.AluOpType.mult, op1=mybir.AluOpType.add,
                )
            nc.scalar.dma_start(y3[t], yt[:])

        # ring software pipeline: `fronts` maps tile index -> (st, rden).
        # The prologue emits fronts 0..AHEAD-1; each body slot t emits
        # front((t+AHEAD)%T) then tail(t). Across For_i iterations the ring
        # closes seamlessly (iteration i's trailing fronts feed iteration
        # i+1's first tails; with repeats==1 they are dead work that
        # overlaps the last tails).
        fronts = {}
        for t in range(min(ahead, m_tiles)):
            fronts[t] = emit_front(t)

        def body(_iv=None):
            for t in range(m_tiles):
                tn = (t + ahead) % m_tiles
                fronts[tn] = emit_front(tn)
                emit_tail(t, *fronts.pop(t))

        if repeats == 1:
            body()
        elif repeats > 1:
            with tc.For_i(0, repeats, 1):
                body()
    nc.finalize()
    return nc


def unpack_weights_host(weight_packed):
    wp = np.asarray(weight_packed)
    parts = [((wp >> (2 * i)) & 3) for i in range(4)]
    w = np.concatenate(parts, axis=0).astype(np.float32) - 1.0   # [out, in]
    return np.ascontiguousarray(w.T).astype(ml_dtypes.bfloat16)  # [in, out]


_NC_CACHE = {}


def _get_nc():
    if "nc" not in _NC_CACHE:
        _NC_CACHE["nc"] = build_nc()
    return _NC_CACHE["nc"]


def make_in_maps(inputs):
    xf = np.ascontiguousarray(
        np.asarray(inputs["x"], dtype=np.float32).reshape(M_TOTAL, D))
    wT = unpack_weights_host(inputs["weight_packed"])
    bias_np = np.ascontiguousarray(np.asarray(inputs["bias"], dtype=np.float32))
    ws_np = np.ascontiguousarray(
        np.asarray(inputs["weight_scale"], dtype=np.float32))
    return [
        {
            "x": xf[i * M_CORE:(i + 1) * M_CORE],
            "wT": wT,
            "bias": bias_np,
            "ws": ws_np,
        }
        for i in range(N_CORES)
    ]


def kernel(x, weight_packed, weight_scale, bias):
    in_maps = make_in_maps(
        {"x": x, "weight_packed": weight_packed,
         "weight_scale": weight_scale, "bias": bias})
    res = run_bass_kernel_spmd(_get_nc(), in_maps, list(range(N_CORES))).results
    y = np.concatenate([res[i]["y"] for i in range(N_CORES)], axis=0)
    return np.ascontiguousarray(y.reshape(B, S, O))

